# revision 2
# baseline (speedup 1.0000x reference)
"""MultiHeadMlp TRN2 kernel: grouped per-head MLP + SE channel attention.

Full-input contract: kernel(**inputs) takes the complete arrays and returns
the complete output. Internally shards data-parallel over the batch dim
(B=8 -> 8 NeuronCores), builds one SPMD Bass/Tile program, and runs it via
run_bass_kernel_spmd.

Math (per batch element b, all tokens local to one core):
    xh = x.reshape(N, H, D)
    h  = gelu(xh @ W1 + b1)          per head, D=256 -> HID=1024
    o  = h @ W2 + b2                 per head, HID   -> D
    out = concat_heads(o)            (N, C)
    pooled = out.mean(axis=0)        (C,)
    gate = sigmoid(relu(pooled@cw1+cb1)@cw2+cb2)
    y = out * (1 + gate)

Layout strategy: activations run channel-major ("transposed") through both
GEMMs, so W1 [D,HID] and W2 [HID,D] serve directly as lhsT and the SE pool
is a free-dim reduction. x is DMA-transposed (bf16) on load; o^T is
DMA-transposed back to token-major while compute continues.
"""

import numpy as np
import ml_dtypes

B = 8
N = 4096
DIM = 1024
H = 4
HD = 256           # head dim
HID = 1024         # per-head hidden
SQ = 64            # squeeze dim
TCH = 512          # tokens per chunk
NCHUNK = N // TCH  # 8
NTOK_TILES = N // 128  # 32
NCORES = 8

_BF = ml_dtypes.bfloat16

_cache = {}


def _build():
    from contextlib import ExitStack

    import concourse.bass as bass
    import concourse.mybir as mybir
    from concourse import bacc
    from concourse.tile import TileContext

    dt = mybir.dt
    bf = dt.bfloat16
    f32 = dt.float32
    Act = mybir.ActivationFunctionType
    Alu = mybir.AluOpType
    Ax = mybir.AxisListType

    nc = bacc.Bacc("TRN2", target_bir_lowering=False, debug=False)

    x = nc.dram_tensor("x", [N, DIM], bf, kind="ExternalInput")
    w1 = nc.dram_tensor("w1", [H, HD, HID], bf, kind="ExternalInput")
    w2 = nc.dram_tensor("w2", [H, HID, HD], bf, kind="ExternalInput")
    b1t = nc.dram_tensor("b1t", [128, H * 8], f32, kind="ExternalInput")
    b2t = nc.dram_tensor("b2t", [128, 8], f32, kind="ExternalInput")
    cw1 = nc.dram_tensor("cw1", [DIM, SQ], bf, kind="ExternalInput")
    cb1t = nc.dram_tensor("cb1t", [SQ, 1], f32, kind="ExternalInput")
    cw2 = nc.dram_tensor("cw2", [SQ, DIM], bf, kind="ExternalInput")
    cb2 = nc.dram_tensor("cb2", [1, DIM], f32, kind="ExternalInput")
    out = nc.dram_tensor("out", [N, DIM], bf, kind="ExternalOutput")

    with TileContext(nc) as tc, ExitStack() as ctx:
        const = ctx.enter_context(tc.tile_pool(name="const", bufs=1))
        xpool = ctx.enter_context(tc.tile_pool(name="xpool", bufs=2))
        hpool = ctx.enter_context(tc.tile_pool(name="hpool", bufs=2))
        otpool = ctx.enter_context(tc.tile_pool(name="otpool", bufs=4))
        ympool = ctx.enter_context(tc.tile_pool(name="ympool", bufs=4))
        pg1 = ctx.enter_context(tc.tile_pool(name="pg1", bufs=3, space="PSUM"))
        pg2 = ctx.enter_context(tc.tile_pool(name="pg2", bufs=3, space="PSUM"))

        # ---- persistent weights / constants ----
        w1sb = {}
        for h in range(H):
            for k in range(2):
                t = const.tile([128, HID], bf, name=f"w1sb_{h}_{k}",
                               tag=f"w1sb_{h}_{k}")
                nc.sync.dma_start(out=t, in_=w1[h, k * 128:(k + 1) * 128, :])
                w1sb[h, k] = t
        w2sb = {}
        for h in range(H):
            for k in range(8):
                t = const.tile([128, HD], bf, name=f"w2sb_{h}_{k}",
                               tag=f"w2sb_{h}_{k}")
                nc.sync.dma_start(out=t, in_=w2[h, k * 128:(k + 1) * 128, :])
                w2sb[h, k] = t
        b1sb = const.tile([128, H * 8], f32, name="b1sb", tag="b1sb")
        nc.sync.dma_start(out=b1sb, in_=b1t[:, :])
        b2sb = const.tile([128, 8], f32, name="b2sb", tag="b2sb")
        nc.sync.dma_start(out=b2sb, in_=b2t[:, :])
        cw1sb = {}
        for c in range(8):
            t = const.tile([128, SQ], bf, name=f"cw1sb_{c}", tag=f"cw1sb_{c}")
            nc.sync.dma_start(out=t, in_=cw1[c * 128:(c + 1) * 128, :])
            cw1sb[c] = t
        cb1sb = const.tile([SQ, 1], f32, name="cb1sb", tag="cb1sb")
        nc.sync.dma_start(out=cb1sb, in_=cb1t[:, :])
        cw2sb = const.tile([SQ, DIM], bf, name="cw2sb", tag="cw2sb")
        nc.sync.dma_start(out=cw2sb, in_=cw2[:, :])
        cb2sb = const.tile([1, DIM], f32, name="cb2sb", tag="cb2sb")
        nc.sync.dma_start(out=cb2sb, in_=cb2[:, :])
        ones1 = const.tile([1, 128], bf, name="ones1", tag="ones1")
        nc.vector.memset(ones1, 1.0)

        # token-major out accumulator (persists across whole kernel)
        otm = []
        for j in range(NTOK_TILES):
            t = const.tile([128, DIM], bf, name=f"otm_{j}", tag=f"otm_{j}")
            otm.append(t)
        # per-(chunk, chan-tile) row sums for the SE pool
        prow = const.tile([128, NCHUNK * 8], f32, name="prow", tag="prow")

        # ---- main loop over token chunks ----
        for i in range(NCHUNK):
            t0 = i * TCH
            xt = []
            for c in range(8):
                t = xpool.tile([128, TCH], bf, name=f"xt_{c}", tag=f"xt_{c}")
                nc.sync.dma_start(
                    out=t,
                    in_=x[t0:t0 + TCH, c * 128:(c + 1) * 128],
                    transpose=True,
                )
                xt.append(t)

            for h in range(H):
                # GEMM1: h^T[m-tile] = gelu(W1_h^T x^T + b1)
                ht = []
                for m in range(8):
                    p1 = pg1.tile([128, TCH], f32, name="p1", tag="p1")
                    nc.tensor.matmul(
                        p1, lhsT=w1sb[h, 0][:, m * 128:(m + 1) * 128],
                        rhs=xt[2 * h], start=True, stop=False)
                    nc.tensor.matmul(
                        p1, lhsT=w1sb[h, 1][:, m * 128:(m + 1) * 128],
                        rhs=xt[2 * h + 1], start=False, stop=True)
                    hm = hpool.tile([128, TCH], bf, name=f"ht_{m}",
                                    tag=f"ht_{m}")
                    nc.scalar.activation(
                        out=hm, in_=p1, func=Act.Gelu,
                        bias=b1sb[:, h * 8 + m:h * 8 + m + 1])
                    ht.append(hm)
                # GEMM2: o^T[d-half] = W2_h^T h^T + b2
                for d in range(2):
                    c = h * 2 + d
                    p2 = pg2.tile([128, TCH], f32, name="p2", tag="p2")
                    for k in range(8):
                        nc.tensor.matmul(
                            p2, lhsT=w2sb[h, k][:, d * 128:(d + 1) * 128],
                            rhs=ht[k], start=(k == 0), stop=(k == 7))
                    ot = otpool.tile([128, TCH], bf, name="ot", tag="ot")
                    nc.vector.tensor_scalar(
                        out=ot, in0=p2, scalar1=b2sb[:, c:c + 1],
                        scalar2=0.0, op0=Alu.add, op1=Alu.add,
                        accum_out=prow[:, i * 8 + c:i * 8 + c + 1])
                    # transpose o^T back to token-major while compute goes on
                    for t in range(4):
                        nc.sync.dma_start(
                            out=otm[i * 4 + t][:, c * 128:(c + 1) * 128],
                            in_=ot[:, t * 128:(t + 1) * 128],
                            transpose=True,
                        )

        # ---- SE channel attention on pooled means ----
        pooled_raw = const.tile([128, 8], f32, name="pooled_raw",
                                tag="pooled_raw")
        prow3 = prow.rearrange("p (i c) -> p i c", c=8)
        for c in range(8):
            nc.vector.tensor_reduce(
                out=pooled_raw[:, c:c + 1], in_=prow3[:, :, c],
                axis=Ax.X, op=Alu.add)
        pooledT = const.tile([128, 8], bf, name="pooledT", tag="pooledT")
        nc.vector.tensor_scalar_mul(pooledT, pooled_raw, 1.0 / N)

        pz = pg1.tile([SQ, 1], f32, name="pz", tag="p1")
        for c in range(8):
            nc.tensor.matmul(pz, lhsT=cw1sb[c], rhs=pooledT[:, c:c + 1],
                             start=(c == 0), stop=(c == 7))
        z1sb = const.tile([SQ, 1], bf, name="z1sb", tag="z1sb")
        nc.scalar.activation(out=z1sb, in_=pz, func=Act.Relu, bias=cb1sb)

        gsb = const.tile([1, DIM], f32, name="gsb", tag="gsb")
        g1sb = const.tile([1, DIM], bf, name="g1sb", tag="g1sb")
        for n in range(2):
            gp = pg2.tile([1, TCH], f32, name="gp", tag="p2")
            nc.tensor.matmul(gp, lhsT=z1sb,
                             rhs=cw2sb[:, n * 512:(n + 1) * 512],
                             start=True, stop=True)
            nc.vector.tensor_tensor(
                out=gsb[:, n * 512:(n + 1) * 512], in0=gp,
                in1=cb2sb[:, n * 512:(n + 1) * 512], op=Alu.add)
            nc.scalar.activation(
                out=gsb[:, n * 512:(n + 1) * 512],
                in_=gsb[:, n * 512:(n + 1) * 512], func=Act.Sigmoid)
            # 1 + gate, in bf16 for the broadcast matmul
            nc.vector.tensor_scalar_add(
                g1sb[:, n * 512:(n + 1) * 512],
                gsb[:, n * 512:(n + 1) * 512], 1.0)

        gb = const.tile([128, DIM], bf, name="gb", tag="gb")
        for n in range(2):
            bp = pg1.tile([128, TCH], f32, name="bp", tag="p1")
            nc.tensor.matmul(bp, lhsT=ones1,
                             rhs=g1sb[:, n * 512:(n + 1) * 512],
                             start=True, stop=True)
            nc.vector.tensor_copy(out=gb[:, n * 512:(n + 1) * 512], in_=bp)

        # ---- final scale + store ----
        for j in range(NTOK_TILES):
            ym = ympool.tile([128, DIM], bf, name="ym", tag="ym")
            nc.vector.tensor_tensor(out=ym, in0=otm[j], in1=gb, op=Alu.mult)
            nc.sync.dma_start(out=out[j * 128:(j + 1) * 128, :], in_=ym)

    nc.compile()
    return nc


def _get_nc():
    if "nc" not in _cache:
        _cache["nc"] = _build()
    return _cache["nc"]


def kernel(x, W1, b1, W2, b2, cw1, cb1, cw2, cb2):
    from concourse.bass_utils import run_bass_kernel_spmd

    nc = _get_nc()

    xb = np.asarray(x, dtype=_BF)               # (B, N, DIM)
    w1b = np.asarray(W1, dtype=_BF)
    w2b = np.asarray(W2, dtype=_BF)
    cw1b = np.asarray(cw1, dtype=_BF)
    cw2b = np.asarray(cw2, dtype=_BF)
    b1tv = np.ascontiguousarray(
        np.asarray(b1, np.float32).reshape(H, 8, 128).transpose(2, 0, 1)
        .reshape(128, H * 8))
    b2tv = np.ascontiguousarray(
        np.asarray(b2, np.float32).reshape(H, 2, 128).transpose(2, 0, 1)
        .reshape(128, 8))
    cb1v = np.asarray(cb1, np.float32).reshape(SQ, 1)
    cb2v = np.asarray(cb2, np.float32).reshape(1, DIM)

    shared = {
        "w1": w1b, "w2": w2b, "b1t": b1tv, "b2t": b2tv,
        "cw1": cw1b, "cb1t": cb1v, "cw2": cw2b, "cb2": cb2v,
    }
    in_maps = [dict(shared, x=np.ascontiguousarray(xb[i]))
               for i in range(NCORES)]

    res = run_bass_kernel_spmd(nc, in_maps, core_ids=list(range(NCORES)))
    y = np.stack([res.results[i]["out"] for i in range(NCORES)], axis=0)
    return y.astype(np.float32)
